# revision 7
# baseline (speedup 1.0000x reference)
"""EnsembleGATDGFLayer Trainium2 kernel.

Data-parallel over batch: 64 graphs -> 8 NeuronCores, 8 graphs each.
All layout prep (transposes, weight folding) happens on host; the device
kernel is pure matmul + elementwise with zero on-chip transposes.

Math (per graph, N=512 nodes, D=256 feat, P=64 op-emb):
  dense = gate_d * (adj @ (X@W)) + X@W + b      (DenseGraphFlow)
  scores = X @ M @ X.T,  M = Wq.T diag(a_w) Wk / 16
  attn = softmax(leaky_relu(scores) * adj)
  gat = LN(gate_g * attn @ (X@Wv.T)) * g + b2   (GraphAttention)
  out = 0.5*(dense + gat)

Key tricks:
  - All matmuls fp32r (tf32; 4x fp32 rate at free-dim >= 256); host
    pre-rounds matmul operands RNE to tf32.
  - adj, exp(scores) and [support|Whv] are carried in bf16: halves the
    LDWEIGHTS time of the AS / h matmuls (their stationaries are 4096 of
    the ~7000 LDW rows per graph) and halves the adj DMA bytes.  The
    PE verifier requires both operands of a matmul to share dtype when
    either is fp32/fp32r, so the moving operand (comb) is bf16 too.
  - scores computed TRANSPOSED [l, e] so adj is only needed transposed
    (host-provided) and attn (=exp, unnormalized) feeds matmuls directly.
  - softmax 1/S normalization is per-row positive -> cancels inside the
    downstream LayerNorm (scale invariance): never computed at all.
  - sigmoid(x) == 0.5*tanh(x/2)+0.5: gates use ACT Tanh so every ACT func
    lives in one act-table set -> no table reloads; the +1/x0.5 factors fold
    into scalar_tensor_tensor consumers and pre-scaled weights.
  - rstd via Quake rsqrt + Newton step (no ACT Sqrt table reload).
  - leaky_relu runs BEFORE the adj mask (valid: adj>=0 commutes with
    leaky) so Prelu reads PSUM on ACT and the mask is SBUF*SBUF, which
    is legal on Pool (GPSIMD cannot access PSUM).
  - engine balance: mask + final residual add + LN scalar chain on Pool,
    big STT/affine/bn on DVE, exp/tanh/prelu + PSUM->SBUF casts on ACT.
  - all matmuls land in [128,2,512] PSUM pair-tiles (2 banks each, 3
    rotating + AS accumulator = 8 banks) so every consumer op covers two
    128-chunks at once -> half the per-op fixed latency.
  - per-graph emission is software-pipelined: front(g+1) before back(g) so
    the PE always has independent matmuls while exp/leaky cook.
"""

import os

import numpy as np

B, N, DIN, DOUT, DOP = 64, 512, 256, 256, 64
NCORES = 8
G = B // NCORES
LN_EPS = 1e-5
NEG = 0.2
QMAGIC = 0x5F3759DF
USE_PRELU = os.environ.get("USE_PRELU", "1") != "0"
BF16 = os.environ.get("BF16", "1") != "0"
POOL_MASK = os.environ.get("POOL_MASK", "1") != "0"
POOL_CHAIN = os.environ.get("POOL_CHAIN", "1") != "0"

_BUILT = {}


def build_bass(g=G, mm_dt_name="float32r", apply_lng=False, use_prelu=None):
    """Build the per-core Bass module processing `g` graphs."""
    if use_prelu is None:
        use_prelu = USE_PRELU
    key = (g, mm_dt_name, apply_lng, use_prelu, BF16, POOL_MASK, POOL_CHAIN)
    if key in _BUILT:
        return _BUILT[key]

    import concourse.bass as bass
    import concourse.tile as tile
    from concourse import bacc, mybir

    f32 = mybir.dt.float32
    i32 = mybir.dt.int32
    bf16 = mybir.dt.bfloat16
    fmm = getattr(mybir.dt, mm_dt_name)
    fadj = bf16 if BF16 else fmm
    fex = bf16 if BF16 else fmm
    fcb = bf16 if BF16 else fmm
    AF = mybir.ActivationFunctionType
    OP = mybir.AluOpType

    nc = bacc.Bacc(None, target_bir_lowering=False, debug=False)

    # -------- DRAM I/O --------
    xt_d = nc.dram_tensor("xt", [g, 2, 128, N], fmm, kind="ExternalInput")
    adjt_d = nc.dram_tensor("adjt", [g, 4, 128, N], fadj, kind="ExternalInput")
    eta_d = nc.dram_tensor("eta", [g, 65, N], fmm, kind="ExternalInput")
    wc_d = nc.dram_tensor("wcomb", [2, 128, 512], fmm, kind="ExternalInput")
    mq_d = nc.dram_tensor("mq", [2, 128, DIN], fmm, kind="ExternalInput")
    go_d = nc.dram_tensor("gcomb", [65, 512], fmm, kind="ExternalInput")
    ch_d = nc.dram_tensor("chalf", [1, 2 * DOUT], f32, kind="ExternalInput")
    lng_d = nc.dram_tensor("lngh", [1, DOUT], f32, kind="ExternalInput")
    out_d = nc.dram_tensor("out", [g, 4, 128, DOUT], f32, kind="ExternalOutput")

    mm = nc.tensor.matmul
    # engine picks for the balance knobs (Pool never touches PSUM)
    e_mask = nc.gpsimd if POOL_MASK else nc.vector
    e_chain = nc.gpsimd if POOL_CHAIN else nc.vector

    with tile.TileContext(nc) as tc:
        with (
            tc.tile_pool(name="const", bufs=1) as cpool,
            tc.tile_pool(name="work", bufs=2) as wpool,
            tc.tile_pool(name="psp", bufs=3, space="PSUM") as psp,
            tc.tile_pool(name="ps2", bufs=1, space="PSUM") as ps2,
        ):
            # -------- replicated params --------
            mq_t = cpool.tile([128, 2, DIN], fmm)
            wc_t = cpool.tile([128, 2, 512], fmm)
            go_t = cpool.tile([65, 512], fmm)
            cb_t = cpool.tile([128, 2 * DOUT], f32)
            if apply_lng:
                lng_t = cpool.tile([128, DOUT], f32)

            def load_consts():
                nc.sync.dma_start(out=mq_t[:],
                                  in_=mq_d[:].rearrange("c p m -> p c m"))
                nc.sync.dma_start(out=wc_t[:],
                                  in_=wc_d[:].rearrange("c p m -> p c m"))
                nc.sync.dma_start(out=go_t[:], in_=go_d[:])
                nc.sync.dma_start(out=cb_t[:],
                                  in_=ch_d[:].to_broadcast([128, 2 * DOUT]))
                if apply_lng:
                    nc.sync.dma_start(out=lng_t[:],
                                      in_=lng_d[:].to_broadcast([128, DOUT]))

            def loads(gi):
                xt = wpool.tile([128, 2, N], fmm, tag="xt", bufs=3)
                nc.sync.dma_start(out=xt[:],
                                  in_=xt_d[gi].rearrange("c p n -> p c n"))
                if gi == 0:
                    load_consts()
                adjt = wpool.tile([128, 4, N], fadj, tag="adjt", bufs=3)
                nc.sync.dma_start(out=adjt[:],
                                  in_=adjt_d[gi].rearrange("c p n -> p c n"))
                eta = wpool.tile([65, N], fmm, tag="eta", bufs=3)
                nc.sync.dma_start(out=eta[:], in_=eta_d[gi])
                return dict(xt=xt, adjt=adjt, eta=eta)

            def fA(gi, st):
                """yt + [support|Whv] projections + scores for graph gi."""
                xt, adjt = st["xt"], st["adjt"]

                # YT = M.T @ XT  [d', e] -- one psum pair, one paired copy
                yt = wpool.tile([128, 2, N], fmm, tag="yt")
                p = psp.tile([128, 2, 512], f32, tag="psp")
                for mc in range(2):
                    for kc in range(2):
                        mm(p[:, mc, :], mq_t[:, kc, mc * 128:(mc + 1) * 128],
                           xt[:, kc, :], start=(kc == 0), stop=(kc == 1))
                nc.scalar.copy(out=yt[:], in_=p[:])

                # [0.25*support | Whv | 2.0 2.0] = X-projections, natural [l, m]
                comb = wpool.tile([128, 4, 516], fcb, tag="comb")
                if BF16:
                    nc.gpsimd.memset(comb[:, :, 512:514], 2.0)
                else:
                    nc.gpsimd.memset(comb[:, :, 512:514].bitcast(f32), 2.0)
                sup_c = wpool.tile([128, 4, DOUT], f32, tag="sup_c")
                cb2 = cb_t[:].rearrange("p (j m) -> p j m", j=2)
                for lh in range(2):
                    p = psp.tile([128, 2, 512], f32, tag="psp")
                    for j in range(2):
                        lc = lh * 2 + j
                        for kc in range(2):
                            mm(p[:, j, :], xt[:, kc, lc * 128:(lc + 1) * 128],
                               wc_t[:, kc, :], start=(kc == 0), stop=(kc == 1))
                    nc.scalar.copy(out=comb[:, lh * 2:lh * 2 + 2, :512],
                                   in_=p[:])
                    # 0.5*support + c == 2*(0.25*support) + c
                    nc.vector.scalar_tensor_tensor(
                        out=sup_c[:, lh * 2:lh * 2 + 2, :],
                        in0=p[:, :, :DOUT], scalar=2.0,
                        in1=cb2, op0=OP.mult, op1=OP.add)

                # scoresT [l, e] = X @ YT ; leaky (PSUM->SBUF) ; mask ; exp
                al = wpool.tile([128, 4, N], f32, tag="al")
                lk = wpool.tile([128, 4, N], f32, tag="lk")
                ex = wpool.tile([128, 4, N], fex, tag="ex")
                for lh in range(2):
                    p = psp.tile([128, 2, 512], f32, tag="psp")
                    for j in range(2):
                        lc = lh * 2 + j
                        for kc in range(2):
                            mm(p[:, j, :], xt[:, kc, lc * 128:(lc + 1) * 128],
                               yt[:, kc, :], start=(kc == 0), stop=(kc == 1))
                    s = slice(lh * 2, lh * 2 + 2)
                    # leaky first: adj >= 0 so leaky(s)*adj == leaky(s*adj)
                    if use_prelu:
                        nc.scalar.activation(out=lk[:, s, :], in_=p[:],
                                             func=AF.Prelu, alpha=NEG)
                    else:
                        nc.vector.scalar_tensor_tensor(
                            out=lk[:, s, :], in0=p[:], scalar=NEG,
                            in1=p[:], op0=OP.mult, op1=OP.max)
                    adj_ap = adjt[:, s, :] if BF16 \
                        else adjt[:, s, :].bitcast(f32)
                    e_mask.tensor_mul(out=al[:, s, :], in0=lk[:, s, :],
                                      in1=adj_ap)
                    nc.scalar.activation(out=ex[:, s, :], in_=al[:, s, :],
                                         func=AF.Exp)

                st.update(yt=yt, comb=comb, sup_c=sup_c, ex=ex)
                return st

            def fB(gi, st):
                """gates for graph gi: [gate_d | gate_g], sigmoid via tanh."""
                eta = st["eta"]
                th = wpool.tile([128, 4, 512], f32, tag="th")
                for eh in range(2):
                    p = psp.tile([128, 2, 512], f32, tag="psp")
                    for j in range(2):
                        ec = eh * 2 + j
                        mm(p[:, j, :], eta[:, ec * 128:(ec + 1) * 128],
                           go_t[:], start=True, stop=True)
                    nc.scalar.activation(out=th[:, eh * 2:eh * 2 + 2, :],
                                         in_=p[:], func=AF.Tanh, scale=0.5)
                st.update(th=th)
                return st

            def bA(gi, st):
                """AS + dense for graph gi."""
                adjt, comb, sup_c, th = st["adjt"], st["comb"], st["sup_c"], st["th"]

                # AS = adjT.T @ (0.25*support), natural [e, m]
                as_ps = ps2.tile([128, 4, DOUT], f32, tag="ps2")
                for ec in range(4):
                    for lc in range(4):
                        mm(as_ps[:, ec, :], adjt[:, lc, ec * 128:(ec + 1) * 128],
                           comb[:, lc, :DOUT], start=(lc == 0), stop=(lc == 3))
                # dense = (tanh_d+1)*AS + (0.5*support + c)
                dn = wpool.tile([128, 4, DOUT], f32, tag="dn")
                nc.vector.scalar_tensor_tensor(
                    out=dn[:], in0=th[:, :, :DOUT], scalar=1.0, in1=as_ps[:],
                    op0=OP.add, op1=OP.mult)
                nc.gpsimd.tensor_add(out=dn[:], in0=dn[:], in1=sup_c[:])
                st.update(dn=dn)
                return st

            def bB(gi, st):
                """attn@Whv + LN + out for graph gi."""
                comb, ex, th, dn = st["comb"], st["ex"], st["th"], st["dn"]
                # v = (tanh_g+1) * (exp @ Whv)  (= 2S * gate_g*attn@Whv; the
                # positive per-row 2S factor cancels in the LayerNorm below,
                # except through eps -- corrected via the 2S column.)
                h = wpool.tile([128, 4, DOUT], f32, tag="h")
                scol = wpool.tile([128, 4, 1], f32, tag="scol")
                for eh in range(2):
                    p = psp.tile([128, 2, 512], f32, tag="psp")
                    for j in range(2):
                        ec = eh * 2 + j
                        for lc in range(4):
                            mm(p[:, j, :258], ex[:, lc, ec * 128:(ec + 1) * 128],
                               comb[:, lc, DOUT:DOUT + 258],
                               start=(lc == 0), stop=(lc == 3))
                    s = slice(eh * 2, eh * 2 + 2)
                    nc.vector.tensor_copy(out=scol[:, s, :],
                                          in_=p[:, :, 256:257])
                    nc.vector.scalar_tensor_tensor(
                        out=h[:, s, :], in0=th[:, s, DOUT:], scalar=1.0,
                        in1=p[:, :, :DOUT], op0=OP.add, op1=OP.mult)

                # LayerNorm over m
                stats = wpool.tile([128, 4, 6], f32, tag="stats")
                mv = wpool.tile([128, 4, 2], f32, tag="mv")
                for ec in range(4):
                    nc.vector.bn_stats(out=stats[:, ec, :], in_=h[:, ec, :])
                    nc.vector.bn_aggr(out=mv[:, ec, :], in_=stats[:, ec, :])
                # rstd (or rstd/2) via Quake rsqrt + Newton step.  Pool
                # cannot run STT or int tensor_scalar ops, so the int seed
                # (w>>1)-QMAGIC is ONE fused DVE op yielding -yq; the
                # negation rides through Newton (odd function) and is
                # unwound in the last two float ops, all Pool-legal.
                # w = sc0*(var_v + eps*(2S)^2); rsqrt(w) absorbs the 2S scale
                w = wpool.tile([128, 4, 1], f32, tag="w")
                s2 = wpool.tile([128, 4, 1], f32, tag="s2")
                sc0 = 1.0 if apply_lng else 4.0
                e_chain.tensor_mul(out=s2[:], in0=scol[:], in1=scol[:])
                e_chain.tensor_scalar(
                    out=w[:], in0=mv[:, :, 1:2], scalar1=sc0,
                    scalar2=None, op0=OP.mult)
                e_chain.tensor_scalar(
                    out=s2[:], in0=s2[:], scalar1=sc0 * LN_EPS,
                    scalar2=None, op0=OP.mult)
                e_chain.tensor_add(out=w[:], in0=w[:], in1=s2[:])
                # yn = -(approx rsqrt(w)) via the int magic: two DVE ops
                # (bitwise and arith ALU stages cannot be fused), emitting
                # (t - QMAGIC) = the INT negation... so negate via *-1 and
                # carry +yq; nb/yq signs below follow the positive branch.
                tq = wpool.tile([128, 4, 1], i32, tag="tq")
                nc.vector.tensor_scalar(
                    out=tq[:], in0=w[:].bitcast(i32), scalar1=1,
                    scalar2=None, op0=OP.arith_shift_right)
                yn = wpool.tile([128, 4, 1], f32, tag="yn")
                nc.vector.tensor_scalar(
                    out=yn[:].bitcast(i32), in0=tq[:], scalar1=QMAGIC,
                    scalar2=-1, op0=OP.subtract, op1=OP.mult)
                aq = wpool.tile([128, 4, 1], f32, tag="aq")
                e_chain.tensor_mul(out=aq[:], in0=yn[:], in1=yn[:])
                e_chain.tensor_scalar(
                    out=aq[:], in0=aq[:], scalar1=-0.5,
                    scalar2=None, op0=OP.mult)
                e_chain.tensor_mul(out=aq[:], in0=aq[:], in1=w[:])
                e_chain.tensor_scalar(
                    out=aq[:], in0=aq[:], scalar1=1.5,
                    scalar2=None, op0=OP.add)
                # Newton: yq = yn*(1.5 - 0.5*w*yn^2) = +rstd
                yq = wpool.tile([128, 4, 1], f32, tag="yq")
                e_chain.tensor_mul(out=yq[:], in0=aq[:], in1=yn[:])
                # nb = -mu*rstd: negate mu on Pool (float imm), then mul
                nb = wpool.tile([128, 4, 1], f32, tag="nb")
                e_chain.tensor_scalar(
                    out=nb[:], in0=mv[:, :, 0:1], scalar1=-1.0,
                    scalar2=None, op0=OP.mult)
                e_chain.tensor_mul(out=nb[:], in0=nb[:], in1=yq[:])
                fin = wpool.tile([128, 4, DOUT], f32, tag="fin")
                if apply_lng:
                    t = wpool.tile([128, 4, DOUT], f32, tag="t")
                    for ec in range(4):
                        nc.scalar.activation(out=t[:, ec, :], in_=h[:, ec, :],
                                             func=AF.Identity,
                                             bias=nb[:, ec, :],
                                             scale=yq[:, ec, :])
                        nc.gpsimd.tensor_mul(out=t[:, ec, :], in0=t[:, ec, :],
                                             in1=lng_t[:])
                    nc.gpsimd.tensor_add(out=fin[:], in0=t[:], in1=dn[:])
                else:
                    # fin = (h*rstd + -mu*rstd) + dense in one DVE op per ec:
                    # sheds the LN-apply from the saturated ACT engine and
                    # the final add from Pool (affine_then_add is HW-proven).
                    for ec in range(4):
                        nc.vector.affine_then_add(
                            out=fin[:, ec, :], in0=h[:, ec, :],
                            in1=dn[:, ec, :],
                            scale=yq[:, ec, :], bias=nb[:, ec, :])
                nc.sync.dma_start(out=out_d[gi].rearrange("c p m -> p c m"),
                                  in_=fin[:])

            # PE warmup: keep the HAM activity monitor busy while the first
            # graph's DMAs land so real matmuls start at full clock.
            wup = cpool.tile([128, N], fmm)
            nc.gpsimd.memset(wup[:].bitcast(f32), 0.25)
            for _ in range(12):
                pw = psp.tile([128, 2, 512], f32, tag="psp")
                mm(pw[:, 0, :], wup[:, :128], wup[:], start=True, stop=True)

            # software pipeline: scores/exp of g+1 get a full loop of
            # runway before the h-matmuls of g+1 consume them.
            #   [fA(g+1): proj+scores+exp] [bA(g): AS] [bB(g): h+LN+out]
            #   [fB(g+1): gates]
            sts = {0: loads(0)}
            if g > 1:
                sts[1] = loads(1)
            fA(0, sts[0])
            fB(0, sts[0])
            for gi in range(1, g):
                if gi + 1 < g:
                    sts[gi + 1] = loads(gi + 1)
                fA(gi, sts[gi])
                bA(gi - 1, sts[gi - 1])
                bB(gi - 1, sts[gi - 1])
                del sts[gi - 1]
                fB(gi, sts[gi])
            bA(g - 1, sts[g - 1])
            bB(g - 1, sts[g - 1])

    nc.compile()
    _BUILT[key] = nc
    return nc


def tf32_round(a):
    """Round-to-nearest-even fp32 -> tf32 (10-bit mantissa) == fp32r."""
    u = np.ascontiguousarray(a, np.float32).view(np.uint32)
    u = (u + np.uint32(0x0FFF) + ((u >> np.uint32(13)) & np.uint32(1))) \
        & np.uint32(0xFFFFE000)
    return u.view(np.float32)


def bf16_round(a):
    """Round-to-nearest-even fp32 -> bf16, returned as ml_dtypes.bfloat16."""
    import ml_dtypes

    return np.ascontiguousarray(a, np.float32).astype(ml_dtypes.bfloat16)


def prep_host(inputs, adj, op_emb, dgf_W, dgf_b, dgf_opW, dgf_opb,
              Wk, Wv, Wq, a_w, gat_opW, gat_opb, ln_g, ln_b):
    """Fold params + lay out per-graph tensors for the device kernel."""
    f = np.float32
    x = np.asarray(inputs, f)
    adj = np.asarray(adj, f)
    ope = np.asarray(op_emb, f)
    nb = x.shape[0]

    xt = np.ascontiguousarray(x.transpose(0, 2, 1)).reshape(nb, 2, 128, N)
    adjt = np.ascontiguousarray(adj.transpose(0, 2, 1)).reshape(nb, 4, 128, N)
    et = np.ascontiguousarray(ope.transpose(0, 2, 1))  # [nb, 64, N]
    eta = np.concatenate([et, np.ones((nb, 1, N), f)], axis=1)  # [nb, 65, N]

    wcomb = np.ascontiguousarray(np.concatenate(
        [0.25 * np.asarray(dgf_W, f), np.asarray(Wv, f).T],
        axis=1)).reshape(2, 128, 512)
    mq = np.ascontiguousarray(
        (np.asarray(Wq, f).T * np.asarray(a_w, f)[None, :]) @ np.asarray(Wk, f)
        / np.sqrt(np.float32(DOUT))).reshape(2, 128, DIN)
    gcomb = np.ascontiguousarray(np.concatenate([
        np.concatenate([np.asarray(dgf_opW, f).T,
                        np.asarray(dgf_opb, f)[None, :]], 0),
        np.concatenate([np.asarray(gat_opW, f).T,
                        np.asarray(gat_opb, f)[None, :]], 0)], axis=1))
    ch1 = (0.5 * (np.asarray(dgf_b, f) + np.asarray(ln_b, f))).reshape(1, DOUT)
    ch = np.ascontiguousarray(np.tile(ch1, (1, 2)))
    lng = np.ascontiguousarray((0.5 * np.asarray(ln_g, f)).reshape(1, DOUT))
    apply_lng = not (np.all(np.asarray(ln_g, f) == 1.0))
    hp = dict(xt=xt, adjt=adjt, eta=eta, wcomb=wcomb, mq=mq, gcomb=gcomb,
              chalf=ch, lngh=lng)
    if MM_DT == "float32r":
        # matmul-feeding tensors must carry fp32r(=tf32)-rounded values
        for k in ("xt", "adjt", "eta", "wcomb", "mq", "gcomb"):
            hp[k] = tf32_round(hp[k])
    if BF16:
        hp["adjt"] = bf16_round(adjt)
    return hp, apply_lng


MM_DT = "float32r"


def run(hp, apply_lng, mm_dt=None, trace=False, **kw):
    from concourse.bass_utils import run_bass_kernel_spmd

    nc = build_bass(G, mm_dt or MM_DT, apply_lng)
    in_maps = []
    for c in range(NCORES):
        sl = slice(c * G, (c + 1) * G)
        m = {k: (v[sl] if k in ("xt", "adjt", "eta") else v)
             for k, v in hp.items()}
        in_maps.append(m)
    res = run_bass_kernel_spmd(nc, in_maps, core_ids=list(range(NCORES)),
                               trace=trace, **kw)
    out = np.concatenate(
        [r["out"].reshape(G, N, DOUT) for r in res.results], axis=0)
    return np.ascontiguousarray(out), res


def kernel(**inputs) -> np.ndarray:
    hp, apply_lng = prep_host(**inputs)
    out, _ = run(hp, apply_lng)
    return out


# revision 8
# speedup vs baseline: 1.0091x; 1.0091x over previous
"""EnsembleGATDGFLayer Trainium2 kernel.

Data-parallel over batch: 64 graphs -> 8 NeuronCores, 8 graphs each.
All layout prep (transposes, weight folding) happens on host; the device
kernel is pure matmul + elementwise with zero on-chip transposes.

Math (per graph, N=512 nodes, D=256 feat, P=64 op-emb):
  dense = gate_d * (adj @ (X@W)) + X@W + b      (DenseGraphFlow)
  scores = X @ M @ X.T,  M = Wq.T diag(a_w) Wk / 16
  attn = softmax(leaky_relu(scores) * adj)
  gat = LN(gate_g * attn @ (X@Wv.T)) * g + b2   (GraphAttention)
  out = 0.5*(dense + gat)

Key tricks:
  - All matmuls fp32r (tf32; 4x fp32 rate at free-dim >= 256); host
    pre-rounds matmul operands RNE to tf32.
  - adj, exp(scores) and [support|Whv] are carried in bf16: halves the
    LDWEIGHTS time of the AS / h matmuls (their stationaries are 4096 of
    the ~7000 LDW rows per graph) and halves the adj DMA bytes.  The
    PE verifier requires both operands of a matmul to share dtype when
    either is fp32/fp32r, so the moving operand (comb) is bf16 too.
  - scores computed TRANSPOSED [l, e] so adj is only needed transposed
    (host-provided) and attn (=exp, unnormalized) feeds matmuls directly.
  - softmax 1/S normalization is per-row positive -> cancels inside the
    downstream LayerNorm (scale invariance): never computed at all.
  - sigmoid(x) == 0.5*tanh(x/2)+0.5: gates use ACT Tanh so every ACT func
    lives in one act-table set -> no table reloads; the +1/x0.5 factors fold
    into scalar_tensor_tensor consumers and pre-scaled weights.
  - rstd via Quake rsqrt + Newton step (no ACT Sqrt table reload).
  - leaky_relu runs BEFORE the adj mask (valid: adj>=0 commutes with
    leaky) so Prelu reads PSUM on ACT and the mask is SBUF*SBUF, which
    is legal on Pool (GPSIMD cannot access PSUM).
  - engine balance: mask + final residual add + LN scalar chain on Pool,
    big STT/affine/bn on DVE, exp/tanh/prelu + PSUM->SBUF casts on ACT.
  - all matmuls land in [128,2,512] PSUM pair-tiles (2 banks each, 3
    rotating + AS accumulator = 8 banks) so every consumer op covers two
    128-chunks at once -> half the per-op fixed latency.
  - per-graph emission is software-pipelined: front(g+1) before back(g) so
    the PE always has independent matmuls while exp/leaky cook.
"""

import os

import numpy as np

B, N, DIN, DOUT, DOP = 64, 512, 256, 256, 64
NCORES = 8
G = B // NCORES
LN_EPS = 1e-5
NEG = 0.2
QMAGIC = 0x5F3759DF
USE_PRELU = os.environ.get("USE_PRELU", "1") != "0"
BF16 = os.environ.get("BF16", "1") != "0"
POOL_MASK = os.environ.get("POOL_MASK", "0") != "0"
POOL_CHAIN = os.environ.get("POOL_CHAIN", "1") != "0"

_BUILT = {}


def build_bass(g=G, mm_dt_name="float32r", apply_lng=False, use_prelu=None):
    """Build the per-core Bass module processing `g` graphs."""
    if use_prelu is None:
        use_prelu = USE_PRELU
    key = (g, mm_dt_name, apply_lng, use_prelu, BF16, POOL_MASK, POOL_CHAIN)
    if key in _BUILT:
        return _BUILT[key]

    import concourse.bass as bass
    import concourse.tile as tile
    from concourse import bacc, mybir

    f32 = mybir.dt.float32
    i32 = mybir.dt.int32
    bf16 = mybir.dt.bfloat16
    fmm = getattr(mybir.dt, mm_dt_name)
    fadj = bf16 if BF16 else fmm
    fex = bf16 if BF16 else fmm
    fcb = bf16 if BF16 else fmm
    AF = mybir.ActivationFunctionType
    OP = mybir.AluOpType

    nc = bacc.Bacc(None, target_bir_lowering=False, debug=False)

    # -------- DRAM I/O --------
    xt_d = nc.dram_tensor("xt", [g, 2, 128, N], fmm, kind="ExternalInput")
    adjt_d = nc.dram_tensor("adjt", [g, 4, 128, N], fadj, kind="ExternalInput")
    eta_d = nc.dram_tensor("eta", [g, 65, N], fmm, kind="ExternalInput")
    wc_d = nc.dram_tensor("wcomb", [2, 128, 512], fmm, kind="ExternalInput")
    mq_d = nc.dram_tensor("mq", [2, 128, DIN], fmm, kind="ExternalInput")
    go_d = nc.dram_tensor("gcomb", [65, 512], fmm, kind="ExternalInput")
    ch_d = nc.dram_tensor("chalf", [1, 2 * DOUT], f32, kind="ExternalInput")
    lng_d = nc.dram_tensor("lngh", [1, DOUT], f32, kind="ExternalInput")
    out_d = nc.dram_tensor("out", [g, 4, 128, DOUT], f32, kind="ExternalOutput")

    mm = nc.tensor.matmul
    # engine picks for the balance knobs (Pool never touches PSUM)
    e_mask = nc.gpsimd if POOL_MASK else nc.vector
    e_chain = nc.gpsimd if POOL_CHAIN else nc.vector

    with tile.TileContext(nc) as tc:
        with (
            tc.tile_pool(name="const", bufs=1) as cpool,
            tc.tile_pool(name="work", bufs=2) as wpool,
            tc.tile_pool(name="psp", bufs=3, space="PSUM") as psp,
            tc.tile_pool(name="ps2", bufs=1, space="PSUM") as ps2,
        ):
            # -------- replicated params --------
            mq_t = cpool.tile([128, 2, DIN], fmm)
            wc_t = cpool.tile([128, 2, 512], fmm)
            go_t = cpool.tile([65, 512], fmm)
            cb_t = cpool.tile([128, 2 * DOUT], f32)
            if apply_lng:
                lng_t = cpool.tile([128, DOUT], f32)

            def loads(gi):
                xt = wpool.tile([128, 2, N], fmm, tag="xt", bufs=3)
                nc.sync.dma_start(out=xt[:],
                                  in_=xt_d[gi].rearrange("c p n -> p c n"))
                if gi == 0:
                    # consts interleaved in first-use order: mq (yt MMs),
                    # wc (comb MMs) first; cb mid; go (gates) last
                    nc.sync.dma_start(out=mq_t[:],
                                      in_=mq_d[:].rearrange("c p m -> p c m"))
                    nc.sync.dma_start(out=wc_t[:],
                                      in_=wc_d[:].rearrange("c p m -> p c m"))
                adjt = wpool.tile([128, 4, N], fadj, tag="adjt", bufs=3)
                nc.sync.dma_start(out=adjt[:],
                                  in_=adjt_d[gi].rearrange("c p n -> p c n"))
                if gi == 0:
                    nc.sync.dma_start(out=cb_t[:],
                                      in_=ch_d[:].to_broadcast([128, 2 * DOUT]))
                eta = wpool.tile([65, N], fmm, tag="eta", bufs=3)
                nc.sync.dma_start(out=eta[:], in_=eta_d[gi])
                if gi == 0:
                    nc.sync.dma_start(out=go_t[:], in_=go_d[:])
                    if apply_lng:
                        nc.sync.dma_start(
                            out=lng_t[:],
                            in_=lng_d[:].to_broadcast([128, DOUT]))
                return dict(xt=xt, adjt=adjt, eta=eta)

            def fA(gi, st):
                """yt + [support|Whv] projections + scores for graph gi."""
                xt, adjt = st["xt"], st["adjt"]

                # YT = M.T @ XT  [d', e] -- one psum pair, one paired copy
                yt = wpool.tile([128, 2, N], fmm, tag="yt")
                p = psp.tile([128, 2, 512], f32, tag="psp")
                for mc in range(2):
                    for kc in range(2):
                        mm(p[:, mc, :], mq_t[:, kc, mc * 128:(mc + 1) * 128],
                           xt[:, kc, :], start=(kc == 0), stop=(kc == 1))
                nc.scalar.copy(out=yt[:], in_=p[:])

                # [0.25*support | Whv | 2.0 2.0] = X-projections, natural [l, m]
                comb = wpool.tile([128, 4, 516], fcb, tag="comb")
                if BF16:
                    nc.gpsimd.memset(comb[:, :, 512:514], 2.0)
                else:
                    nc.gpsimd.memset(comb[:, :, 512:514].bitcast(f32), 2.0)
                sup_c = wpool.tile([128, 4, DOUT], f32, tag="sup_c")
                cb2 = cb_t[:].rearrange("p (j m) -> p j m", j=2)
                for lh in range(2):
                    p = psp.tile([128, 2, 512], f32, tag="psp")
                    for j in range(2):
                        lc = lh * 2 + j
                        for kc in range(2):
                            mm(p[:, j, :], xt[:, kc, lc * 128:(lc + 1) * 128],
                               wc_t[:, kc, :], start=(kc == 0), stop=(kc == 1))
                    nc.scalar.copy(out=comb[:, lh * 2:lh * 2 + 2, :512],
                                   in_=p[:])
                    # 0.5*support + c == 2*(0.25*support) + c
                    nc.vector.scalar_tensor_tensor(
                        out=sup_c[:, lh * 2:lh * 2 + 2, :],
                        in0=p[:, :, :DOUT], scalar=2.0,
                        in1=cb2, op0=OP.mult, op1=OP.add)

                # scoresT [l, e] = X @ YT ; leaky (PSUM->SBUF) ; mask ; exp
                al = wpool.tile([128, 4, N], f32, tag="al")
                lk = wpool.tile([128, 4, N], f32, tag="lk")
                ex = wpool.tile([128, 4, N], fex, tag="ex")
                for lh in range(2):
                    p = psp.tile([128, 2, 512], f32, tag="psp")
                    for j in range(2):
                        lc = lh * 2 + j
                        for kc in range(2):
                            mm(p[:, j, :], xt[:, kc, lc * 128:(lc + 1) * 128],
                               yt[:, kc, :], start=(kc == 0), stop=(kc == 1))
                    s = slice(lh * 2, lh * 2 + 2)
                    # leaky first: adj >= 0 so leaky(s)*adj == leaky(s*adj)
                    if use_prelu:
                        nc.scalar.activation(out=lk[:, s, :], in_=p[:],
                                             func=AF.Prelu, alpha=NEG)
                    else:
                        nc.vector.scalar_tensor_tensor(
                            out=lk[:, s, :], in0=p[:], scalar=NEG,
                            in1=p[:], op0=OP.mult, op1=OP.max)
                    adj_ap = adjt[:, s, :] if BF16 \
                        else adjt[:, s, :].bitcast(f32)
                    e_mask.tensor_mul(out=al[:, s, :], in0=lk[:, s, :],
                                      in1=adj_ap)
                    nc.scalar.activation(out=ex[:, s, :], in_=al[:, s, :],
                                         func=AF.Exp)

                st.update(yt=yt, comb=comb, sup_c=sup_c, ex=ex)
                return st

            def fB(gi, st):
                """gates for graph gi: [gate_d | gate_g], sigmoid via tanh."""
                eta = st["eta"]
                th = wpool.tile([128, 4, 512], f32, tag="th")
                for eh in range(2):
                    p = psp.tile([128, 2, 512], f32, tag="psp")
                    for j in range(2):
                        ec = eh * 2 + j
                        mm(p[:, j, :], eta[:, ec * 128:(ec + 1) * 128],
                           go_t[:], start=True, stop=True)
                    nc.scalar.activation(out=th[:, eh * 2:eh * 2 + 2, :],
                                         in_=p[:], func=AF.Tanh, scale=0.5)
                st.update(th=th)
                return st

            def bA(gi, st):
                """AS + dense for graph gi."""
                adjt, comb, sup_c, th = st["adjt"], st["comb"], st["sup_c"], st["th"]

                # AS = adjT.T @ (0.25*support), natural [e, m]
                as_ps = ps2.tile([128, 4, DOUT], f32, tag="ps2")
                for ec in range(4):
                    for lc in range(4):
                        mm(as_ps[:, ec, :], adjt[:, lc, ec * 128:(ec + 1) * 128],
                           comb[:, lc, :DOUT], start=(lc == 0), stop=(lc == 3))
                # dense = (tanh_d+1)*AS + (0.5*support + c)
                dn = wpool.tile([128, 4, DOUT], f32, tag="dn")
                nc.vector.scalar_tensor_tensor(
                    out=dn[:], in0=th[:, :, :DOUT], scalar=1.0, in1=as_ps[:],
                    op0=OP.add, op1=OP.mult)
                nc.gpsimd.tensor_add(out=dn[:], in0=dn[:], in1=sup_c[:])
                st.update(dn=dn)
                return st

            def bB(gi, st):
                """attn@Whv + LN + out for graph gi."""
                comb, ex, th, dn = st["comb"], st["ex"], st["th"], st["dn"]
                # v = (tanh_g+1) * (exp @ Whv)  (= 2S * gate_g*attn@Whv; the
                # positive per-row 2S factor cancels in the LayerNorm below,
                # except through eps -- corrected via the 2S column.)
                h = wpool.tile([128, 4, DOUT], f32, tag="h")
                scol = wpool.tile([128, 4, 1], f32, tag="scol")
                for eh in range(2):
                    p = psp.tile([128, 2, 512], f32, tag="psp")
                    for j in range(2):
                        ec = eh * 2 + j
                        for lc in range(4):
                            mm(p[:, j, :258], ex[:, lc, ec * 128:(ec + 1) * 128],
                               comb[:, lc, DOUT:DOUT + 258],
                               start=(lc == 0), stop=(lc == 3))
                    s = slice(eh * 2, eh * 2 + 2)
                    nc.vector.tensor_copy(out=scol[:, s, :],
                                          in_=p[:, :, 256:257])
                    nc.vector.scalar_tensor_tensor(
                        out=h[:, s, :], in0=th[:, s, DOUT:], scalar=1.0,
                        in1=p[:, :, :DOUT], op0=OP.add, op1=OP.mult)

                # LayerNorm over m
                stats = wpool.tile([128, 4, 6], f32, tag="stats")
                mv = wpool.tile([128, 4, 2], f32, tag="mv")
                for ec in range(4):
                    nc.vector.bn_stats(out=stats[:, ec, :], in_=h[:, ec, :])
                    nc.vector.bn_aggr(out=mv[:, ec, :], in_=stats[:, ec, :])
                # rstd (or rstd/2) via Quake rsqrt + Newton step.  Pool
                # cannot run STT or int tensor_scalar ops, so the int seed
                # (w>>1)-QMAGIC is ONE fused DVE op yielding -yq; the
                # negation rides through Newton (odd function) and is
                # unwound in the last two float ops, all Pool-legal.
                # w = sc0*(var_v + eps*(2S)^2); rsqrt(w) absorbs the 2S scale
                w = wpool.tile([128, 4, 1], f32, tag="w")
                s2 = wpool.tile([128, 4, 1], f32, tag="s2")
                sc0 = 1.0 if apply_lng else 4.0
                e_chain.tensor_mul(out=s2[:], in0=scol[:], in1=scol[:])
                e_chain.tensor_scalar(
                    out=w[:], in0=mv[:, :, 1:2], scalar1=sc0,
                    scalar2=None, op0=OP.mult)
                e_chain.tensor_scalar(
                    out=s2[:], in0=s2[:], scalar1=sc0 * LN_EPS,
                    scalar2=None, op0=OP.mult)
                e_chain.tensor_add(out=w[:], in0=w[:], in1=s2[:])
                # yn = -(approx rsqrt(w)) via the int magic: two DVE ops
                # (bitwise and arith ALU stages cannot be fused), emitting
                # (t - QMAGIC) = the INT negation... so negate via *-1 and
                # carry +yq; nb/yq signs below follow the positive branch.
                tq = wpool.tile([128, 4, 1], i32, tag="tq")
                nc.vector.tensor_scalar(
                    out=tq[:], in0=w[:].bitcast(i32), scalar1=1,
                    scalar2=None, op0=OP.arith_shift_right)
                yn = wpool.tile([128, 4, 1], f32, tag="yn")
                nc.vector.tensor_scalar(
                    out=yn[:].bitcast(i32), in0=tq[:], scalar1=QMAGIC,
                    scalar2=-1, op0=OP.subtract, op1=OP.mult)
                aq = wpool.tile([128, 4, 1], f32, tag="aq")
                e_chain.tensor_mul(out=aq[:], in0=yn[:], in1=yn[:])
                e_chain.tensor_scalar(
                    out=aq[:], in0=aq[:], scalar1=-0.5,
                    scalar2=None, op0=OP.mult)
                e_chain.tensor_mul(out=aq[:], in0=aq[:], in1=w[:])
                e_chain.tensor_scalar(
                    out=aq[:], in0=aq[:], scalar1=1.5,
                    scalar2=None, op0=OP.add)
                # Newton: yq = yn*(1.5 - 0.5*w*yn^2) = +rstd
                yq = wpool.tile([128, 4, 1], f32, tag="yq")
                e_chain.tensor_mul(out=yq[:], in0=aq[:], in1=yn[:])
                # nb = -mu*rstd: negate mu on Pool (float imm), then mul
                nb = wpool.tile([128, 4, 1], f32, tag="nb")
                e_chain.tensor_scalar(
                    out=nb[:], in0=mv[:, :, 0:1], scalar1=-1.0,
                    scalar2=None, op0=OP.mult)
                e_chain.tensor_mul(out=nb[:], in0=nb[:], in1=yq[:])
                fin = wpool.tile([128, 4, DOUT], f32, tag="fin")
                if apply_lng:
                    t = wpool.tile([128, 4, DOUT], f32, tag="t")
                    for ec in range(4):
                        nc.scalar.activation(out=t[:, ec, :], in_=h[:, ec, :],
                                             func=AF.Identity,
                                             bias=nb[:, ec, :],
                                             scale=yq[:, ec, :])
                        nc.gpsimd.tensor_mul(out=t[:, ec, :], in0=t[:, ec, :],
                                             in1=lng_t[:])
                    nc.gpsimd.tensor_add(out=fin[:], in0=t[:], in1=dn[:])
                else:
                    # fin = (h*rstd + -mu*rstd) + dense in one DVE op per ec:
                    # sheds the LN-apply from the saturated ACT engine and
                    # the final add from Pool (affine_then_add is HW-proven).
                    # per-ec DMA: each is a 128KB linear DRAM block and
                    # starts as soon as its affine lands (fast tail drain)
                    for ec in range(4):
                        nc.vector.affine_then_add(
                            out=fin[:, ec, :], in0=h[:, ec, :],
                            in1=dn[:, ec, :],
                            scale=yq[:, ec, :], bias=nb[:, ec, :])
                        nc.sync.dma_start(out=out_d[gi, ec],
                                          in_=fin[:, ec, :])
                    return
                nc.sync.dma_start(out=out_d[gi].rearrange("c p m -> p c m"),
                                  in_=fin[:])

            # PE warmup: keep the HAM activity monitor busy while the first
            # graph's DMAs land so real matmuls start at full clock.
            wup = cpool.tile([128, N], fmm)
            nc.gpsimd.memset(wup[:].bitcast(f32), 0.25)
            for _ in range(12):
                pw = psp.tile([128, 2, 512], f32, tag="psp")
                mm(pw[:, 0, :], wup[:, :128], wup[:], start=True, stop=True)

            # software pipeline: scores/exp of g+1 get a full loop of
            # runway before the h-matmuls of g+1 consume them.
            #   [fA(g+1): proj+scores+exp] [bA(g): AS] [bB(g): h+LN+out]
            #   [fB(g+1): gates]
            sts = {0: loads(0)}
            if g > 1:
                sts[1] = loads(1)
            fA(0, sts[0])
            fB(0, sts[0])
            for gi in range(1, g):
                if gi + 1 < g:
                    sts[gi + 1] = loads(gi + 1)
                fA(gi, sts[gi])
                bA(gi - 1, sts[gi - 1])
                bB(gi - 1, sts[gi - 1])
                del sts[gi - 1]
                fB(gi, sts[gi])
            bA(g - 1, sts[g - 1])
            bB(g - 1, sts[g - 1])

    nc.compile()
    _BUILT[key] = nc
    return nc


def tf32_round(a):
    """Round-to-nearest-even fp32 -> tf32 (10-bit mantissa) == fp32r."""
    u = np.ascontiguousarray(a, np.float32).view(np.uint32)
    u = (u + np.uint32(0x0FFF) + ((u >> np.uint32(13)) & np.uint32(1))) \
        & np.uint32(0xFFFFE000)
    return u.view(np.float32)


def bf16_round(a):
    """Round-to-nearest-even fp32 -> bf16, returned as ml_dtypes.bfloat16."""
    import ml_dtypes

    return np.ascontiguousarray(a, np.float32).astype(ml_dtypes.bfloat16)


def prep_host(inputs, adj, op_emb, dgf_W, dgf_b, dgf_opW, dgf_opb,
              Wk, Wv, Wq, a_w, gat_opW, gat_opb, ln_g, ln_b):
    """Fold params + lay out per-graph tensors for the device kernel."""
    f = np.float32
    x = np.asarray(inputs, f)
    adj = np.asarray(adj, f)
    ope = np.asarray(op_emb, f)
    nb = x.shape[0]

    xt = np.ascontiguousarray(x.transpose(0, 2, 1)).reshape(nb, 2, 128, N)
    adjt = np.ascontiguousarray(adj.transpose(0, 2, 1)).reshape(nb, 4, 128, N)
    et = np.ascontiguousarray(ope.transpose(0, 2, 1))  # [nb, 64, N]
    eta = np.concatenate([et, np.ones((nb, 1, N), f)], axis=1)  # [nb, 65, N]

    wcomb = np.ascontiguousarray(np.concatenate(
        [0.25 * np.asarray(dgf_W, f), np.asarray(Wv, f).T],
        axis=1)).reshape(2, 128, 512)
    mq = np.ascontiguousarray(
        (np.asarray(Wq, f).T * np.asarray(a_w, f)[None, :]) @ np.asarray(Wk, f)
        / np.sqrt(np.float32(DOUT))).reshape(2, 128, DIN)
    gcomb = np.ascontiguousarray(np.concatenate([
        np.concatenate([np.asarray(dgf_opW, f).T,
                        np.asarray(dgf_opb, f)[None, :]], 0),
        np.concatenate([np.asarray(gat_opW, f).T,
                        np.asarray(gat_opb, f)[None, :]], 0)], axis=1))
    ch1 = (0.5 * (np.asarray(dgf_b, f) + np.asarray(ln_b, f))).reshape(1, DOUT)
    ch = np.ascontiguousarray(np.tile(ch1, (1, 2)))
    lng = np.ascontiguousarray((0.5 * np.asarray(ln_g, f)).reshape(1, DOUT))
    apply_lng = not (np.all(np.asarray(ln_g, f) == 1.0))
    hp = dict(xt=xt, adjt=adjt, eta=eta, wcomb=wcomb, mq=mq, gcomb=gcomb,
              chalf=ch, lngh=lng)
    if MM_DT == "float32r":
        # matmul-feeding tensors must carry fp32r(=tf32)-rounded values
        for k in ("xt", "adjt", "eta", "wcomb", "mq", "gcomb"):
            hp[k] = tf32_round(hp[k])
    if BF16:
        hp["adjt"] = bf16_round(adjt)
    return hp, apply_lng


MM_DT = "float32r"


def run(hp, apply_lng, mm_dt=None, trace=False, **kw):
    from concourse.bass_utils import run_bass_kernel_spmd

    nc = build_bass(G, mm_dt or MM_DT, apply_lng)
    in_maps = []
    for c in range(NCORES):
        sl = slice(c * G, (c + 1) * G)
        m = {k: (v[sl] if k in ("xt", "adjt", "eta") else v)
             for k, v in hp.items()}
        in_maps.append(m)
    res = run_bass_kernel_spmd(nc, in_maps, core_ids=list(range(NCORES)),
                               trace=trace, **kw)
    out = np.concatenate(
        [r["out"].reshape(G, N, DOUT) for r in res.results], axis=0)
    return np.ascontiguousarray(out), res


def kernel(**inputs) -> np.ndarray:
    hp, apply_lng = prep_host(**inputs)
    out, _ = run(hp, apply_lng)
    return out


# revision 9
# speedup vs baseline: 1.0176x; 1.0084x over previous
"""EnsembleGATDGFLayer Trainium2 kernel.

Data-parallel over batch: 64 graphs -> 8 NeuronCores, 8 graphs each.
All layout prep (transposes, weight folding) happens on host; the device
kernel is pure matmul + elementwise with zero on-chip transposes.

Math (per graph, N=512 nodes, D=256 feat, P=64 op-emb):
  dense = gate_d * (adj @ (X@W)) + X@W + b      (DenseGraphFlow)
  scores = X @ M @ X.T,  M = Wq.T diag(a_w) Wk / 16
  attn = softmax(leaky_relu(scores) * adj)
  gat = LN(gate_g * attn @ (X@Wv.T)) * g + b2   (GraphAttention)
  out = 0.5*(dense + gat)

Key tricks:
  - All matmuls fp32r (tf32; 4x fp32 rate at free-dim >= 256); host
    pre-rounds matmul operands RNE to tf32.
  - adj, exp(scores) and [support|Whv] are carried in bf16: halves the
    LDWEIGHTS time of the AS / h matmuls (their stationaries are 4096 of
    the ~7000 LDW rows per graph) and halves the adj DMA bytes.  The
    PE verifier requires both operands of a matmul to share dtype when
    either is fp32/fp32r, so the moving operand (comb) is bf16 too.
  - scores computed TRANSPOSED [l, e] so adj is only needed transposed
    (host-provided) and attn (=exp, unnormalized) feeds matmuls directly.
  - softmax 1/S normalization is per-row positive -> cancels inside the
    downstream LayerNorm (scale invariance): never computed at all.
  - sigmoid(x) == 0.5*tanh(x/2)+0.5: gates use ACT Tanh so every ACT func
    lives in one act-table set -> no table reloads; the +1/x0.5 factors fold
    into scalar_tensor_tensor consumers and pre-scaled weights.
  - rstd via Quake rsqrt + Newton step (no ACT Sqrt table reload).
  - leaky_relu runs BEFORE the adj mask (valid: adj>=0 commutes with
    leaky) so Prelu reads PSUM on ACT and the mask is SBUF*SBUF, which
    is legal on Pool (GPSIMD cannot access PSUM).
  - engine balance: mask + final residual add + LN scalar chain on Pool,
    big STT/affine/bn on DVE, exp/tanh/prelu + PSUM->SBUF casts on ACT.
  - all matmuls land in [128,2,512] PSUM pair-tiles (2 banks each, 3
    rotating + AS accumulator = 8 banks) so every consumer op covers two
    128-chunks at once -> half the per-op fixed latency.
  - per-graph emission is software-pipelined: front(g+1) before back(g) so
    the PE always has independent matmuls while exp/leaky cook.
"""

import os

import numpy as np

B, N, DIN, DOUT, DOP = 64, 512, 256, 256, 64
NCORES = 8
G = B // NCORES
LN_EPS = 1e-5
NEG = 0.2
QMAGIC = 0x5F3759DF
USE_PRELU = os.environ.get("USE_PRELU", "1") != "0"
BF16 = os.environ.get("BF16", "1") != "0"
POOL_MASK = os.environ.get("POOL_MASK", "1") != "0"
POOL_CHAIN = os.environ.get("POOL_CHAIN", "1") != "0"

_BUILT = {}


def build_bass(g=G, mm_dt_name="float32r", apply_lng=False, use_prelu=None):
    """Build the per-core Bass module processing `g` graphs."""
    if use_prelu is None:
        use_prelu = USE_PRELU
    key = (g, mm_dt_name, apply_lng, use_prelu, BF16, POOL_MASK, POOL_CHAIN)
    if key in _BUILT:
        return _BUILT[key]

    import concourse.bass as bass
    import concourse.tile as tile
    from concourse import bacc, mybir

    f32 = mybir.dt.float32
    i32 = mybir.dt.int32
    bf16 = mybir.dt.bfloat16
    fmm = getattr(mybir.dt, mm_dt_name)
    fadj = bf16 if BF16 else fmm
    fex = bf16 if BF16 else fmm
    fcb = bf16 if BF16 else fmm
    AF = mybir.ActivationFunctionType
    OP = mybir.AluOpType

    nc = bacc.Bacc(None, target_bir_lowering=False, debug=False)

    # -------- DRAM I/O --------
    xt_d = nc.dram_tensor("xt", [g, 2, 128, N], fmm, kind="ExternalInput")
    adjt_d = nc.dram_tensor("adjt", [g, 4, 128, N], fadj, kind="ExternalInput")
    eta_d = nc.dram_tensor("eta", [g, 65, N], fmm, kind="ExternalInput")
    wc_d = nc.dram_tensor("wcomb", [2, 128, 512], fmm, kind="ExternalInput")
    mq_d = nc.dram_tensor("mq", [2, 128, DIN], fmm, kind="ExternalInput")
    go_d = nc.dram_tensor("gcomb", [65, 512], fmm, kind="ExternalInput")
    ch_d = nc.dram_tensor("chalf", [1, 2 * DOUT], f32, kind="ExternalInput")
    lng_d = nc.dram_tensor("lngh", [1, DOUT], f32, kind="ExternalInput")
    out_d = nc.dram_tensor("out", [g, 4, 128, DOUT], f32, kind="ExternalOutput")

    mm = nc.tensor.matmul
    # engine picks for the balance knobs (Pool never touches PSUM)
    e_mask = nc.gpsimd if POOL_MASK else nc.vector
    e_chain = nc.gpsimd if POOL_CHAIN else nc.vector

    with tile.TileContext(nc) as tc:
        with (
            tc.tile_pool(name="const", bufs=1) as cpool,
            tc.tile_pool(name="work", bufs=2) as wpool,
            tc.tile_pool(name="psp", bufs=3, space="PSUM") as psp,
            tc.tile_pool(name="ps2", bufs=1, space="PSUM") as ps2,
        ):
            # -------- replicated params --------
            mq_t = cpool.tile([128, 2, DIN], fmm)
            wc_t = cpool.tile([128, 2, 512], fmm)
            go_t = cpool.tile([65, 512], fmm)
            cb_t = cpool.tile([128, 2 * DOUT], f32)
            if apply_lng:
                lng_t = cpool.tile([128, DOUT], f32)

            def loads(gi):
                xt = wpool.tile([128, 2, N], fmm, tag="xt", bufs=3)
                nc.sync.dma_start(out=xt[:],
                                  in_=xt_d[gi].rearrange("c p n -> p c n"))
                if gi == 0:
                    # consts interleaved in first-use order: mq (yt MMs),
                    # wc (comb MMs) first; cb mid; go (gates) last
                    nc.sync.dma_start(out=mq_t[:],
                                      in_=mq_d[:].rearrange("c p m -> p c m"))
                    nc.sync.dma_start(out=wc_t[:],
                                      in_=wc_d[:].rearrange("c p m -> p c m"))
                adjt = wpool.tile([128, 4, N], fadj, tag="adjt", bufs=3)
                nc.sync.dma_start(out=adjt[:],
                                  in_=adjt_d[gi].rearrange("c p n -> p c n"))
                if gi == 0:
                    nc.sync.dma_start(out=cb_t[:],
                                      in_=ch_d[:].to_broadcast([128, 2 * DOUT]))
                eta = wpool.tile([65, N], fmm, tag="eta", bufs=3)
                nc.sync.dma_start(out=eta[:], in_=eta_d[gi])
                if gi == 0:
                    nc.sync.dma_start(out=go_t[:], in_=go_d[:])
                    if apply_lng:
                        nc.sync.dma_start(
                            out=lng_t[:],
                            in_=lng_d[:].to_broadcast([128, DOUT]))
                return dict(xt=xt, adjt=adjt, eta=eta)

            def fA(gi, st):
                """yt + [support|Whv] projections + scores for graph gi."""
                xt, adjt = st["xt"], st["adjt"]

                # YT = M.T @ XT  [d', e] -- one psum pair, one paired copy
                yt = wpool.tile([128, 2, N], fmm, tag="yt")
                p = psp.tile([128, 2, 512], f32, tag="psp")
                for mc in range(2):
                    for kc in range(2):
                        mm(p[:, mc, :], mq_t[:, kc, mc * 128:(mc + 1) * 128],
                           xt[:, kc, :], start=(kc == 0), stop=(kc == 1))
                nc.scalar.copy(out=yt[:], in_=p[:])

                # [0.25*support | Whv | 2.0 2.0] = X-projections, natural [l, m]
                comb = wpool.tile([128, 4, 516], fcb, tag="comb")
                if BF16:
                    nc.gpsimd.memset(comb[:, :, 512:514], 2.0)
                else:
                    nc.gpsimd.memset(comb[:, :, 512:514].bitcast(f32), 2.0)
                sup_c = wpool.tile([128, 4, DOUT], f32, tag="sup_c")
                cb2 = cb_t[:].rearrange("p (j m) -> p j m", j=2)
                for lh in range(2):
                    p = psp.tile([128, 2, 512], f32, tag="psp")
                    for j in range(2):
                        lc = lh * 2 + j
                        for kc in range(2):
                            mm(p[:, j, :], xt[:, kc, lc * 128:(lc + 1) * 128],
                               wc_t[:, kc, :], start=(kc == 0), stop=(kc == 1))
                    nc.scalar.copy(out=comb[:, lh * 2:lh * 2 + 2, :512],
                                   in_=p[:])
                    # 0.5*support + c == 2*(0.25*support) + c
                    nc.vector.scalar_tensor_tensor(
                        out=sup_c[:, lh * 2:lh * 2 + 2, :],
                        in0=p[:, :, :DOUT], scalar=2.0,
                        in1=cb2, op0=OP.mult, op1=OP.add)

                # scoresT [l, e] = X @ YT ; leaky (PSUM->SBUF on ACT)
                lk = wpool.tile([128, 4, N], f32, tag="lk")
                for lh in range(2):
                    p = psp.tile([128, 2, 512], f32, tag="psp")
                    for j in range(2):
                        lc = lh * 2 + j
                        for kc in range(2):
                            mm(p[:, j, :], xt[:, kc, lc * 128:(lc + 1) * 128],
                               yt[:, kc, :], start=(kc == 0), stop=(kc == 1))
                    s = slice(lh * 2, lh * 2 + 2)
                    # leaky first: adj >= 0 so leaky(s)*adj == leaky(s*adj)
                    if use_prelu:
                        nc.scalar.activation(out=lk[:, s, :], in_=p[:],
                                             func=AF.Prelu, alpha=NEG)
                    else:
                        nc.vector.scalar_tensor_tensor(
                            out=lk[:, s, :], in0=p[:], scalar=NEG,
                            in1=p[:], op0=OP.mult, op1=OP.max)

                st.update(yt=yt, comb=comb, sup_c=sup_c, lk=lk)
                return st

            def fA2(gi, st):
                """mask (Pool, SBUF*SBUF) + exp for graph gi; emitted after
                bB(gi-1) so the LN chain isn't queued behind the masks."""
                adjt, lk = st["adjt"], st["lk"]
                al = wpool.tile([128, 4, N], f32, tag="al")
                ex = wpool.tile([128, 4, N], fex, tag="ex")
                for lh in range(2):
                    s = slice(lh * 2, lh * 2 + 2)
                    adj_ap = adjt[:, s, :] if BF16 \
                        else adjt[:, s, :].bitcast(f32)
                    e_mask.tensor_mul(out=al[:, s, :], in0=lk[:, s, :],
                                      in1=adj_ap)
                    nc.scalar.activation(out=ex[:, s, :], in_=al[:, s, :],
                                         func=AF.Exp)
                st.update(ex=ex)
                return st

            def fB(gi, st):
                """gates for graph gi: [gate_d | gate_g], sigmoid via tanh."""
                eta = st["eta"]
                th = wpool.tile([128, 4, 512], f32, tag="th")
                for eh in range(2):
                    p = psp.tile([128, 2, 512], f32, tag="psp")
                    for j in range(2):
                        ec = eh * 2 + j
                        mm(p[:, j, :], eta[:, ec * 128:(ec + 1) * 128],
                           go_t[:], start=True, stop=True)
                    nc.scalar.activation(out=th[:, eh * 2:eh * 2 + 2, :],
                                         in_=p[:], func=AF.Tanh, scale=0.5)
                st.update(th=th)
                return st

            def bA(gi, st):
                """AS + dense for graph gi."""
                adjt, comb, sup_c, th = st["adjt"], st["comb"], st["sup_c"], st["th"]

                # AS = adjT.T @ (0.25*support), natural [e, m]
                as_ps = ps2.tile([128, 4, DOUT], f32, tag="ps2")
                for ec in range(4):
                    for lc in range(4):
                        mm(as_ps[:, ec, :], adjt[:, lc, ec * 128:(ec + 1) * 128],
                           comb[:, lc, :DOUT], start=(lc == 0), stop=(lc == 3))
                # dense = (tanh_d+1)*AS + (0.5*support + c)
                dn = wpool.tile([128, 4, DOUT], f32, tag="dn")
                nc.vector.scalar_tensor_tensor(
                    out=dn[:], in0=th[:, :, :DOUT], scalar=1.0, in1=as_ps[:],
                    op0=OP.add, op1=OP.mult)
                nc.gpsimd.tensor_add(out=dn[:], in0=dn[:], in1=sup_c[:])
                st.update(dn=dn)
                return st

            def bB(gi, st):
                """attn@Whv + LN + out for graph gi."""
                comb, ex, th, dn = st["comb"], st["ex"], st["th"], st["dn"]
                # v = (tanh_g+1) * (exp @ Whv)  (= 2S * gate_g*attn@Whv; the
                # positive per-row 2S factor cancels in the LayerNorm below,
                # except through eps -- corrected via the 2S column.)
                h = wpool.tile([128, 4, DOUT], f32, tag="h")
                scol = wpool.tile([128, 4, 1], f32, tag="scol")
                for eh in range(2):
                    p = psp.tile([128, 2, 512], f32, tag="psp")
                    for j in range(2):
                        ec = eh * 2 + j
                        for lc in range(4):
                            mm(p[:, j, :258], ex[:, lc, ec * 128:(ec + 1) * 128],
                               comb[:, lc, DOUT:DOUT + 258],
                               start=(lc == 0), stop=(lc == 3))
                    s = slice(eh * 2, eh * 2 + 2)
                    nc.vector.tensor_copy(out=scol[:, s, :],
                                          in_=p[:, :, 256:257])
                    nc.vector.scalar_tensor_tensor(
                        out=h[:, s, :], in0=th[:, s, DOUT:], scalar=1.0,
                        in1=p[:, :, :DOUT], op0=OP.add, op1=OP.mult)

                # LayerNorm over m
                stats = wpool.tile([128, 4, 6], f32, tag="stats")
                mv = wpool.tile([128, 4, 2], f32, tag="mv")
                for ec in range(4):
                    nc.vector.bn_stats(out=stats[:, ec, :], in_=h[:, ec, :])
                    nc.vector.bn_aggr(out=mv[:, ec, :], in_=stats[:, ec, :])
                # rstd (or rstd/2) via Quake rsqrt + Newton step.  Pool
                # cannot run STT or int tensor_scalar ops, so the int seed
                # (w>>1)-QMAGIC is ONE fused DVE op yielding -yq; the
                # negation rides through Newton (odd function) and is
                # unwound in the last two float ops, all Pool-legal.
                # w = sc0*(var_v + eps*(2S)^2); rsqrt(w) absorbs the 2S scale
                w = wpool.tile([128, 4, 1], f32, tag="w")
                s2 = wpool.tile([128, 4, 1], f32, tag="s2")
                sc0 = 1.0 if apply_lng else 4.0
                e_chain.tensor_mul(out=s2[:], in0=scol[:], in1=scol[:])
                e_chain.tensor_scalar(
                    out=w[:], in0=mv[:, :, 1:2], scalar1=sc0,
                    scalar2=None, op0=OP.mult)
                e_chain.tensor_scalar(
                    out=s2[:], in0=s2[:], scalar1=sc0 * LN_EPS,
                    scalar2=None, op0=OP.mult)
                e_chain.tensor_add(out=w[:], in0=w[:], in1=s2[:])
                # yn = -(approx rsqrt(w)) via the int magic: two DVE ops
                # (bitwise and arith ALU stages cannot be fused), emitting
                # (t - QMAGIC) = the INT negation... so negate via *-1 and
                # carry +yq; nb/yq signs below follow the positive branch.
                tq = wpool.tile([128, 4, 1], i32, tag="tq")
                nc.vector.tensor_scalar(
                    out=tq[:], in0=w[:].bitcast(i32), scalar1=1,
                    scalar2=None, op0=OP.arith_shift_right)
                yn = wpool.tile([128, 4, 1], f32, tag="yn")
                nc.vector.tensor_scalar(
                    out=yn[:].bitcast(i32), in0=tq[:], scalar1=QMAGIC,
                    scalar2=-1, op0=OP.subtract, op1=OP.mult)
                aq = wpool.tile([128, 4, 1], f32, tag="aq")
                e_chain.tensor_mul(out=aq[:], in0=yn[:], in1=yn[:])
                e_chain.tensor_scalar(
                    out=aq[:], in0=aq[:], scalar1=-0.5,
                    scalar2=None, op0=OP.mult)
                e_chain.tensor_mul(out=aq[:], in0=aq[:], in1=w[:])
                e_chain.tensor_scalar(
                    out=aq[:], in0=aq[:], scalar1=1.5,
                    scalar2=None, op0=OP.add)
                # Newton: yq = yn*(1.5 - 0.5*w*yn^2) = +rstd
                yq = wpool.tile([128, 4, 1], f32, tag="yq")
                e_chain.tensor_mul(out=yq[:], in0=aq[:], in1=yn[:])
                # nb = -mu*rstd: negate mu on Pool (float imm), then mul
                nb = wpool.tile([128, 4, 1], f32, tag="nb")
                e_chain.tensor_scalar(
                    out=nb[:], in0=mv[:, :, 0:1], scalar1=-1.0,
                    scalar2=None, op0=OP.mult)
                e_chain.tensor_mul(out=nb[:], in0=nb[:], in1=yq[:])
                fin = wpool.tile([128, 4, DOUT], f32, tag="fin")
                if apply_lng:
                    t = wpool.tile([128, 4, DOUT], f32, tag="t")
                    for ec in range(4):
                        nc.scalar.activation(out=t[:, ec, :], in_=h[:, ec, :],
                                             func=AF.Identity,
                                             bias=nb[:, ec, :],
                                             scale=yq[:, ec, :])
                        nc.gpsimd.tensor_mul(out=t[:, ec, :], in0=t[:, ec, :],
                                             in1=lng_t[:])
                    nc.gpsimd.tensor_add(out=fin[:], in0=t[:], in1=dn[:])
                else:
                    # fin = (h*rstd + -mu*rstd) + dense in one DVE op per ec:
                    # sheds the LN-apply from the saturated ACT engine and
                    # the final add from Pool (affine_then_add is HW-proven).
                    # per-ec DMA: each is a 128KB linear DRAM block and
                    # starts as soon as its affine lands (fast tail drain)
                    for ec in range(4):
                        nc.vector.affine_then_add(
                            out=fin[:, ec, :], in0=h[:, ec, :],
                            in1=dn[:, ec, :],
                            scale=yq[:, ec, :], bias=nb[:, ec, :])
                        nc.sync.dma_start(out=out_d[gi, ec],
                                          in_=fin[:, ec, :])
                    return
                nc.sync.dma_start(out=out_d[gi].rearrange("c p m -> p c m"),
                                  in_=fin[:])

            # PE warmup: keep the HAM activity monitor busy while the first
            # graph's DMAs land so real matmuls start at full clock.
            wup = cpool.tile([128, N], fmm)
            nc.gpsimd.memset(wup[:].bitcast(f32), 0.25)
            for _ in range(12):
                pw = psp.tile([128, 2, 512], f32, tag="psp")
                mm(pw[:, 0, :], wup[:, :128], wup[:], start=True, stop=True)

            # software pipeline: scores/exp of g+1 get a full loop of
            # runway before the h-matmuls of g+1 consume them.
            #   [fA(g+1): proj+scores+exp] [bA(g): AS] [bB(g): h+LN+out]
            #   [fB(g+1): gates]
            sts = {0: loads(0)}
            if g > 1:
                sts[1] = loads(1)
            fA(0, sts[0])
            fA2(0, sts[0])
            fB(0, sts[0])
            for gi in range(1, g):
                if gi + 1 < g:
                    sts[gi + 1] = loads(gi + 1)
                fA(gi, sts[gi])
                bA(gi - 1, sts[gi - 1])
                bB(gi - 1, sts[gi - 1])
                del sts[gi - 1]
                fA2(gi, sts[gi])
                fB(gi, sts[gi])
            bA(g - 1, sts[g - 1])
            bB(g - 1, sts[g - 1])

    nc.compile()
    _BUILT[key] = nc
    return nc


def tf32_round(a):
    """Round-to-nearest-even fp32 -> tf32 (10-bit mantissa) == fp32r."""
    u = np.ascontiguousarray(a, np.float32).view(np.uint32)
    u = (u + np.uint32(0x0FFF) + ((u >> np.uint32(13)) & np.uint32(1))) \
        & np.uint32(0xFFFFE000)
    return u.view(np.float32)


def bf16_round(a):
    """Round-to-nearest-even fp32 -> bf16, returned as ml_dtypes.bfloat16."""
    import ml_dtypes

    return np.ascontiguousarray(a, np.float32).astype(ml_dtypes.bfloat16)


def prep_host(inputs, adj, op_emb, dgf_W, dgf_b, dgf_opW, dgf_opb,
              Wk, Wv, Wq, a_w, gat_opW, gat_opb, ln_g, ln_b):
    """Fold params + lay out per-graph tensors for the device kernel."""
    f = np.float32
    x = np.asarray(inputs, f)
    adj = np.asarray(adj, f)
    ope = np.asarray(op_emb, f)
    nb = x.shape[0]

    xt = np.ascontiguousarray(x.transpose(0, 2, 1)).reshape(nb, 2, 128, N)
    adjt = np.ascontiguousarray(adj.transpose(0, 2, 1)).reshape(nb, 4, 128, N)
    et = np.ascontiguousarray(ope.transpose(0, 2, 1))  # [nb, 64, N]
    eta = np.concatenate([et, np.ones((nb, 1, N), f)], axis=1)  # [nb, 65, N]

    wcomb = np.ascontiguousarray(np.concatenate(
        [0.25 * np.asarray(dgf_W, f), np.asarray(Wv, f).T],
        axis=1)).reshape(2, 128, 512)
    mq = np.ascontiguousarray(
        (np.asarray(Wq, f).T * np.asarray(a_w, f)[None, :]) @ np.asarray(Wk, f)
        / np.sqrt(np.float32(DOUT))).reshape(2, 128, DIN)
    gcomb = np.ascontiguousarray(np.concatenate([
        np.concatenate([np.asarray(dgf_opW, f).T,
                        np.asarray(dgf_opb, f)[None, :]], 0),
        np.concatenate([np.asarray(gat_opW, f).T,
                        np.asarray(gat_opb, f)[None, :]], 0)], axis=1))
    ch1 = (0.5 * (np.asarray(dgf_b, f) + np.asarray(ln_b, f))).reshape(1, DOUT)
    ch = np.ascontiguousarray(np.tile(ch1, (1, 2)))
    lng = np.ascontiguousarray((0.5 * np.asarray(ln_g, f)).reshape(1, DOUT))
    apply_lng = not (np.all(np.asarray(ln_g, f) == 1.0))
    hp = dict(xt=xt, adjt=adjt, eta=eta, wcomb=wcomb, mq=mq, gcomb=gcomb,
              chalf=ch, lngh=lng)
    if MM_DT == "float32r":
        # matmul-feeding tensors must carry fp32r(=tf32)-rounded values
        for k in ("xt", "adjt", "eta", "wcomb", "mq", "gcomb"):
            hp[k] = tf32_round(hp[k])
    if BF16:
        hp["adjt"] = bf16_round(adjt)
    return hp, apply_lng


MM_DT = "float32r"


def run(hp, apply_lng, mm_dt=None, trace=False, **kw):
    from concourse.bass_utils import run_bass_kernel_spmd

    nc = build_bass(G, mm_dt or MM_DT, apply_lng)
    in_maps = []
    for c in range(NCORES):
        sl = slice(c * G, (c + 1) * G)
        m = {k: (v[sl] if k in ("xt", "adjt", "eta") else v)
             for k, v in hp.items()}
        in_maps.append(m)
    res = run_bass_kernel_spmd(nc, in_maps, core_ids=list(range(NCORES)),
                               trace=trace, **kw)
    out = np.concatenate(
        [r["out"].reshape(G, N, DOUT) for r in res.results], axis=0)
    return np.ascontiguousarray(out), res


def kernel(**inputs) -> np.ndarray:
    hp, apply_lng = prep_host(**inputs)
    out, _ = run(hp, apply_lng)
    return out
